# revision 63
# baseline (speedup 1.0000x reference)
"""Trainium2 Bass kernel for nn_FCNetwork3D (batch-1 dense CNN+MLP).

Network: x[1,2264] -> 6x Conv3d(1,1,3,SAME)+ReLU on the 6x6x6 tail ->
concat -> normalize -> Linear(2264,4096)+tanh -> Linear(4096,4096)+tanh
-> Linear(4096,32) -> scale/shift.

Sharding (8 cores): tensor-parallel on the two wide Linears.
  L0 column-parallel: core k computes h0 block k [512] (weights pre-
    transposed + normalization folded on host), tanh locally.
  AllGather h0 (1KB/core bf16) on-device.
  L1 column-parallel: core k computes h1 block k [512], tanh locally.
  L2 row-parallel over h1 blocks: core k computes a partial [1,32]
    (out_scale folded into weights, bias/out_shift split /8 across
    cores); host unshard = sum of the 8 partials.

Precision: weights/activations bf16 (fp32 PSUM accumulate), and the
dominant a1 stream (W1.T block, 2M params/core) in fp8-e3m4 with one
bf16-rounded dequant scale per core block, folded into the h1 tanh
(bias pre-divided).  Measured end-to-end max rel err 1.22e-2 vs the
fp32 reference (gate 2e-2); hardware matches the numpy quantization
model exactly.  e4m3 fails (2.7e-2); quantizing a0 too fails (1.97e-2).

The conv stack uses the z-banded form of the 3x3x3 SAME conv on the
6x6x6 grid: 3 matrices [36,36] per layer (one per z-offset) acting on
v viewed as [36 xy-sites, 6 z-slices] — 45KB of conv weights instead
of 1.1MB of dense [216,216] matrices.

Pipelining (reps>1): on this platform each collective_compute costs
~70us of serialized wall time and utterly dominates the un-batched
per-rep cost (measured: removing the collective makes 15 extra reps
free).  So reps are processed in groups of G: phase A runs G conv+L0
blocks, ONE AllGather moves all G h0 blocks (payload is tiny either
way), then phase B runs G [L1,L2] blocks.  Groups are software-
pipelined with full skew — emission order is
  A_0, g_0, A_1, B_0, g_1, A_2, B_1, ... , g_{K-1}, B_{K-1}
— so gather k overlaps A_{k+1}'s compute + DMA, and the weight
streams (a0 in phase A, a1 in phase B) alternate on each DMA queue in
exactly the order they are consumed.

DMA plan: one light queue (sync) carries the small latency-critical
tensors (conv weights, packs, biases, gather in/out rows, y); three
bulk queues (scalar, vector, gpsimd) each carry ~2MB/rep of a0+a1 in
need-order.
"""

import numpy as np

import concourse.bass as bass
import concourse.mybir as mybir
import concourse.tile as tile
from concourse import bacc
from concourse import bass_utils

F32 = mybir.dt.float32
BF16 = mybir.dt.bfloat16
F8 = mybir.dt.float8e3      # e3m4: 4 mantissa bits; a1 weight stream
AF = mybir.ActivationFunctionType

NCORES = 8
OBS, ACTD, H, VOX = 2264, 32, 4096, 216
XH = OBS - VOX            # 2048 (x head)
S = H // NCORES           # 512 (per-core block of the hidden dim)
KC0 = XH // 128           # 16 x-head K-chunks
KC1 = H // 128            # 32 h0 K-chunks
NS = 36                   # xy-sites per z-slice
NZ = 6                    # z-slices
CTW = 6 * 3 * NS          # packed banded conv width (648)

# pkb (bf16 [128, 566] pack) column map: x chunks, voxel tail, then the
# phase-A bias row (1.0 + bias0) rides along in partition 0
PB_X = 0                  # [0:16)    x head, partition-major chunks
PB_V = 16                 # [16:22)   voxel tail as [36, 6]
BA_ONE = 22               # [22]      1.0 (partition 0)
BA_B0 = 54                # [54:566)  bias0 block (partition 0)
PB_W = 566
# ctb carries the conv bias columns (bf16) after the 18 band matrices
CF_B = CTW                # [648:654) per-layer conv bias, broadcast rows
CT_W = CTW + 6
# pkbB (bf16 [1, 576], partition 0): 1.0, fp8 dequant scale, bias1
# block (pre-divided by the scale) at 32..544, bias2 at 544
BB_ONE = 0
BB_SC = 1
BB_B1 = 32
BB_B2 = 544
BB_W = 576

# engine roles: sync + gpsimd issue only prefetch DMA (no compute, no
# dependency waits -> deep in-order lookahead); scalar runs the serial
# activation/copy chains plus the few dependency-gated DMAs (ccin,
# ccrow, y); PE computes; DVE paces the warmup/bridge chains.
A1Q = (8, 12, 12)         # a1 K-chunk split across (sync, gpsimd, gpsimd)


def build_nc(reps: int = 1, fake_gather: bool = False, G: int = 8):
    """Build the per-core Bass program (identical on all 8 cores; data
    differs via per-core inputs). reps>1 unrolls the whole body in
    gather-batched groups of up to G. fake_gather replaces the
    AllGather with a DRAM round-trip (single-core TimelineSim oracle /
    HW no-collective A/B)."""
    nc = bacc.Bacc("TRN2", target_bir_lowering=False, debug=False,
                   num_devices=1 if fake_gather else NCORES)

    ctb_d = nc.dram_tensor("ctb", [NS, CT_W], BF16, kind="ExternalInput")
    pkb_d = nc.dram_tensor("pkb", [128, PB_W], BF16, kind="ExternalInput")
    pkbB_d = nc.dram_tensor("pkbB", [1, BB_W], BF16, kind="ExternalInput")
    a0_d = nc.dram_tensor("a0", [128, KC0 * S], BF16, kind="ExternalInput")
    wtail_d = nc.dram_tensor("wtail", [NS, NZ * S], BF16, kind="ExternalInput")
    a1_d = nc.dram_tensor("a1", [128, KC1 * S], F8, kind="ExternalInput")
    a2_d = nc.dram_tensor("a2", [128, (S // 128) * ACTD], BF16, kind="ExternalInput")
    y_d = nc.dram_tensor("y", [1, ACTD], F32, kind="ExternalOutput")

    scr = nc.alloc_sbuf_tensor("warm_scr", [1, 64], BF16)

    groups = []
    r = 0
    while r < reps:
        groups.append(list(range(r, min(r + G, reps))))
        r += G

    with tile.TileContext(nc) as tc:
        with (
            tc.tile_pool(name="wa", bufs=3) as wa,
            tc.tile_pool(name="wb", bufs=2) as wb,
            tc.tile_pool(name="cp", bufs=2) as cp,
            tc.tile_pool(name="sp", bufs=2) as sp,
            tc.tile_pool(name="ps", bufs=1, space="PSUM") as ps,
            tc.tile_pool(name="dr", bufs=2, space="DRAM") as dr,
        ):
            def emit_A_rep(ccin, i, warmup: bool):
                """One phase-A rep: stage streams + conv stack + L0 +
                tanh + write h0s into the group's ccin row i."""
                if True:
                    first_group = warmup
                    # --- light HWDGE smalls, in conv/L0 need-order ---
                    # (deep-buffered: these gate in-order queues at rep
                    # boundaries if their slots recycle too slowly)
                    pkb = sp.tile([128, PB_W], BF16, name="pkb", bufs=4)
                    nc.sync.dma_start(out=pkb[:], in_=pkb_d.ap())
                    ctb = cp.tile([NS, CT_W], BF16, name="ctb", bufs=4)
                    nc.sync.dma_start(out=ctb[:], in_=ctb_d.ap())
                    wtail = cp.tile([NS, NZ * S], BF16, name="wtail", bufs=4)
                    nc.gpsimd.dma_start(out=wtail[:], in_=wtail_d.ap())

                    # --- a0 stream: one 2MB prefetch on sync ---
                    a0t = wa.tile([128, KC0 * S], BF16, name="a0t")
                    nc.sync.dma_start(out=a0t[:], in_=a0_d[:, :])

                    if first_group:
                        # tensor-clock warmup from t=0: self-paced PE<->DVE
                        # chain on an uninitialized scratch tensor.
                        pwm = ps.tile([1, 64], F32, name="ptr2", bufs=2)
                        wseed = sp.tile([1, 1], BF16, name="brb")
                        nc.vector.tensor_copy(wseed[:], scr.ap()[0:1, 0:1])
                        for w in range(6):
                            nc.tensor.matmul(pwm[:], wseed[:], scr.ap()[0:1, :],
                                             start=(w == 0), stop=(w == 5),
                                             skip_group_check=True)
                            if w < 5:
                                wseed = sp.tile([1, 1], BF16, name="brb")
                                nc.vector.tensor_copy(wseed[:], pwm[0:1, 0:1])

                    # --- conv stack + L0, interleaved ---
                    # The 6 conv relu round-trips (PE -> scalar -> PE)
                    # would stall the in-order PE ~600ns each; instead
                    # the L0 x-chunk matmuls are dealt between conv
                    # layers so PE streams weights while each relu
                    # flies.  The conv-dependent z-tail matmuls close
                    # the ph0 accumulation.
                    hp = tc.high_priority()
                    hp.__enter__()
                    one_a = pkb[0:1, BA_ONE:BA_ONE + 1]
                    ph0 = ps.tile([1, S], F32, name="ph0", bufs=2)
                    nc.tensor.matmul(ph0[:], one_a,
                                     pkb[0:1, BA_B0:BA_B0 + S],
                                     start=True, stop=False)
                    XBATCH = (3, 3, 3, 3, 2, 2)
                    v = pkb[0:NS, PB_V:PB_V + NZ]
                    c = 0
                    for li in range(6):
                        b = li * 3 * NS
                        pm = ps.tile([NS, NZ], F32, name="pm", bufs=2)
                        nc.tensor.matmul(pm[:, 0:6], ctb[:, b + 36:b + 72],
                                         v[:, 0:6], start=True, stop=False,
                                         skip_group_check=True)
                        nc.tensor.matmul(pm[:, 1:6], ctb[:, b:b + 36],
                                         v[:, 0:5], start=False, stop=False,
                                         skip_group_check=True)
                        nc.tensor.matmul(pm[:, 0:5], ctb[:, b + 72:b + 108],
                                         v[:, 1:6], start=False, stop=True,
                                         skip_group_check=True)
                        nv = sp.tile([NS, NZ], BF16, name="nv")
                        nc.scalar.activation(nv[:], pm[:], AF.Relu,
                                             bias=ctb[:, CF_B + li:CF_B + li + 1])
                        v = nv[:]
                        for _ in range(XBATCH[li]):
                            nc.tensor.matmul(ph0[:], pkb[:, c:c + 1],
                                             a0t[:, c * S:(c + 1) * S],
                                             start=False, stop=False)
                            c += 1
                    for z in range(NZ):
                        nc.tensor.matmul(ph0[:], v[:, z:z + 1],
                                         wtail[:, z * S:(z + 1) * S],
                                         start=False, stop=(z == NZ - 1))
                    h0s = sp.tile([1, S], BF16, name="h0s")
                    nc.scalar.activation(h0s[:], ph0[:], AF.Tanh)
                    nc.scalar.dma_start(out=ccin[i:i + 1, :], in_=h0s[:])
                    hp.__exit__(None, None, None)

            def emit_gather(ccin, gl):
                ccout = dr.tile([gl * H], BF16, name="ccout",
                                addr_space="Local" if fake_gather else "Shared")
                if fake_gather:
                    # stand-in: a DRAM bounce keeps the dependency chain
                    # identical to the real AllGather
                    nc.gpsimd.dma_start(
                        out=ccout[0:gl * S],
                        in_=ccin[:].rearrange("o e -> (o e)"))
                else:
                    nc.gpsimd.collective_compute(
                        "AllGather", mybir.AluOpType.bypass,
                        replica_groups=[list(range(NCORES))],
                        ins=[ccin[:].rearrange("o e -> (o e)").opt()],
                        outs=[ccout[:].opt()])
                return ccout

            def emit_B_rep(ccout, gl, ii):
                """One phase-B rep: fetch+transpose h0, L1, L2 partial,
                y.  (No p-state bridge: with batched gathers the PE
                never idles long enough to down-clock, and the chain
                serialized the in-order engines against real work.)"""
                if True:
                    pkbB = sp.tile([1, BB_W], BF16, name="pkbB")
                    nc.sync.dma_start(out=pkbB[:], in_=pkbB_d.ap())
                    a2t = sp.tile([128, (S // 128) * ACTD], BF16, name="a2t")
                    nc.sync.dma_start(out=a2t[:], in_=a2_d[:, :])
                    one_b = pkbB[0:1, BB_ONE:BB_ONE + 1]
                    # fp8 dequant scale, widened to f32 for the Act unit
                    sc32 = sp.tile([1, 1], F32, name="sc32")
                    nc.vector.tensor_copy(sc32[:], pkbB[0:1, BB_SC:BB_SC + 1])

                    # this rep's gathered h0 row: one strided DMA pulling
                    # the 8 [1,512] core-blocks out of the group buffer
                    ccrow = sp.tile([1, H], BF16, name="ccrow")
                    nc.scalar.dma_start(
                        out=ccrow[:],
                        in_=ccout.rearrange("(c g) -> c g", c=NCORES)
                                 [:, ii * S:(ii + 1) * S])

                    # --- a1 stream: sync + 2x gpsimd prefetches ---
                    a1eng = [nc.sync, nc.gpsimd, nc.gpsimd]
                    a1t = []
                    c1 = 0
                    for q, gch in enumerate(A1Q):
                        wt = wb.tile([128, gch * S], F8, name=f"a1q{q}")
                        a1eng[q].dma_start(
                            out=wt[:], in_=a1_d[:, c1 * S:(c1 + gch) * S])
                        a1t.append(wt)
                        c1 += gch

                    # --- h0 row -> K-chunk columns (PE transposes) ---
                    h0g = sp.tile([128, KC1], BF16, name="h0g")
                    for t in range(4):
                        ptr2 = ps.tile([128, 8], F32, name="ptr2", bufs=2)
                        for j in range(8):
                            cc = t * 8 + j
                            nc.tensor.matmul(
                                ptr2[:, j:j + 1],
                                ccrow[:, cc * 128:(cc + 1) * 128], one_b,
                                start=True, stop=True)
                        nc.scalar.copy(h0g[:, t * 8:(t + 1) * 8], ptr2[:])

                    # --- L1: h1_blk = tanh(h0 @ A1 + b1_blk) [1,512] ---
                    ph1 = ps.tile([1, S], F32, name="ph1")
                    nc.tensor.matmul(ph1[:], one_b,
                                     pkbB[0:1, BB_B1:BB_B1 + S],
                                     start=True, stop=False)
                    # consume the gpsimd tiles (q1, q2) first; sync's q0
                    # trails the next group's a0 stream on its queue, so
                    # its chunks go last in the accumulation
                    qbase = [0, A1Q[0], A1Q[0] + A1Q[1]]
                    qorder = [0, 1, 2]
                    n = 0
                    for q in qorder:
                        for j in range(A1Q[q]):
                            cc = qbase[q] + j
                            n += 1
                            nc.tensor.matmul(ph1[:], h0g[:, cc:cc + 1],
                                             a1t[q][:, j * S:(j + 1) * S],
                                             start=False, stop=(n == KC1))
                    h1s = sp.tile([1, S], BF16, name="h1s")
                    nc.scalar.activation(h1s[:], ph1[:], AF.Tanh,
                                         scale=sc32[:])

                    # --- L2 partial: y_k = h1_blk @ A2_blk + b'/8 ---
                    pth = ps.tile([128, S // 128], F32, name="ptr2", bufs=2)
                    for ccq in range(S // 128):
                        nc.tensor.matmul(pth[:, ccq:ccq + 1],
                                         h1s[:, ccq * 128:(ccq + 1) * 128],
                                         one_b, start=True, stop=True)
                    h1g = sp.tile([128, S // 128], BF16, name="h1g")
                    nc.scalar.copy(h1g[:], pth[:])
                    py = ps.tile([1, ACTD], F32, name="py")
                    nc.tensor.matmul(py[:], one_b,
                                     pkbB[0:1, BB_B2:BB_B2 + ACTD],
                                     start=True, stop=False)
                    for ccq in range(S // 128):
                        nc.tensor.matmul(py[:], h1g[:, ccq:ccq + 1],
                                         a2t[:, ccq * ACTD:(ccq + 1) * ACTD],
                                         start=False,
                                         stop=(ccq == S // 128 - 1))
                    ys = sp.tile([1, ACTD], F32, name="ys")
                    nc.scalar.copy(ys[:], py[:])
                    nc.sync.dma_start(out=y_d[:, :], in_=ys[:])

            # ---- software pipeline over groups, rep-interleaved:
            # A_{k+1} rep i and B_k rep i alternate, so the a0 and a1
            # streams hit the shared DMA pool in 1:1 need-order ----
            gl0 = len(groups[0])
            ccin = dr.tile([gl0, S], BF16, name="ccin")
            for i in range(gl0):
                emit_A_rep(ccin, i, warmup=(i == 0))
            pend = (emit_gather(ccin, gl0), gl0)
            for k in range(1, len(groups)):
                glk = len(groups[k])
                ccin = dr.tile([glk, S], BF16, name="ccin")
                pout, pgl = pend
                for i in range(max(glk, pgl)):
                    if i < glk:
                        emit_A_rep(ccin, i, warmup=False)
                    if i < pgl:
                        emit_B_rep(pout, pgl, i)
                pend = (emit_gather(ccin, glk), glk)
            pout, pgl = pend
            for ii in range(pgl):
                emit_B_rep(pout, pgl, ii)

    nc.compile()
    return nc


def _conv_matrix(w: np.ndarray) -> np.ndarray:
    """[216,216] dense matrix of a 3x3x3 SAME cross-correlation on a
    6x6x6 grid: C[o, i] such that y.flat = C @ v.flat."""
    w = np.asarray(w, dtype=np.float32).reshape(3, 3, 3)
    C = np.zeros((VOX, VOX), dtype=np.float32)
    idx = np.arange(6)
    for dz in (-1, 0, 1):
        for dy in (-1, 0, 1):
            for dx in (-1, 0, 1):
                zo, zi = idx[max(0, -dz):6 - max(0, dz)], idx[max(0, dz):6 - max(0, -dz)]
                yo, yi = idx[max(0, -dy):6 - max(0, dy)], idx[max(0, dy):6 - max(0, -dy)]
                xo, xi = idx[max(0, -dx):6 - max(0, dx)], idx[max(0, dx):6 - max(0, -dx)]
                o = (zo[:, None, None] * 36 + yo[None, :, None] * 6 + xo[None, None, :]).ravel()
                i = (zi[:, None, None] * 36 + yi[None, :, None] * 6 + xi[None, None, :]).ravel()
                C[o, i] = w[dz + 1, dy + 1, dx + 1]
    return C


def make_in_maps(inputs: dict) -> list[dict]:
    """Host-side layout prep + sharding: fold normalization into A0,
    out_scale/shift into A2, pre-transpose weights, build banded conv
    matrices, quantize everything to bf16."""
    import ml_dtypes
    f = np.float32
    bf = ml_dtypes.bfloat16
    f8 = mybir.dt.np(F8)
    F8MAX = float(ml_dtypes.finfo(f8).max)
    x = np.asarray(inputs["x"], f)
    W0, b0 = np.asarray(inputs["W0"], f), np.asarray(inputs["b0"], f)
    W1, b1 = np.asarray(inputs["W1"], f), np.asarray(inputs["b1"], f)
    W2, b2 = np.asarray(inputs["W2"], f), np.asarray(inputs["b2"], f)
    in_shift = np.asarray(inputs["in_shift"], f)
    in_scale = np.asarray(inputs["in_scale"], f)
    out_shift = np.asarray(inputs["out_shift"], f)
    out_scale = np.asarray(inputs["out_scale"], f)

    sc = (1.0 / (in_scale.astype(np.float64) + 1e-8)).astype(f)       # [2264]
    A0 = (W0 * sc[None, :]).T.astype(f)                               # [2264, 4096]
    bias0 = (b0 - (in_shift * sc) @ W0.T).astype(f)                   # [4096]
    A1 = W1.T.astype(f)                                               # [4096, 4096]
    A2 = (W2.T * out_scale[None, :]).astype(f)                        # [4096, 32]
    bias2 = ((b2 * out_scale + out_shift) / NCORES).astype(f)         # [32]

    # banded conv: per layer i and z-offset dz, M_dz [36,36] stored
    # transposed (lhsT layout): ctb[s_in, (i*3 + dz+1)*36 + s_out];
    # per-layer conv biases ride along as broadcast columns at CF_B
    ctb = np.zeros((NS, CT_W), f)
    for i in range(6):
        C = _conv_matrix(inputs[f"cw{i}"])
        for dzi, dz in enumerate((-1, 0, 1)):
            M = C[1 * NS:2 * NS, (1 + dz) * NS:(2 + dz) * NS]         # [out, in]
            ctb[:, (i * 3 + dzi) * NS:(i * 3 + dzi + 1) * NS] = M.T
    cb = np.array([np.asarray(inputs[f"cb{i}"], f).ravel()[0]
                   for i in range(6)], f)
    ctb[:, CF_B:CF_B + 6] = np.broadcast_to(cb[None, :], (NS, 6))

    xh = x.ravel()[:XH]
    v0 = x.ravel()[XH:]
    pkb = np.zeros((128, PB_W), f)
    pkb[:, PB_X:PB_X + KC0] = xh.reshape(KC0, 128).T
    pkb[0:NS, PB_V:PB_V + NZ] = v0.reshape(NZ, NS).T
    pkb[0, BA_ONE] = 1.0  # bias0 row is filled per-core below

    # A0 tail columns regrouped by z-slice: wtail[s, z*S+n] = A0[2048+z*36+s, n]
    def wtail_for(blk):
        t = A0[XH:OBS, blk].reshape(NZ, NS, S)
        return np.ascontiguousarray(t.transpose(1, 0, 2).reshape(NS, NZ * S))

    in_maps = []
    for k in range(NCORES):
        blk = slice(k * S, (k + 1) * S)
        pkbk = pkb.copy()
        pkbk[0, BA_B0:BA_B0 + S] = bias0[blk]
        a1blk = A1[:, blk]
        # fp8-e3m4 a1: one bf16-rounded dequant scale per core block,
        # folded into the h1 tanh (bias pre-divided so it scales back)
        s1 = np.float32(np.asarray(np.abs(a1blk).max() / F8MAX, bf))
        pkbB = np.zeros((1, BB_W), f)
        pkbB[0, BB_ONE] = 1.0
        pkbB[0, BB_SC] = s1
        pkbB[0, BB_B1:BB_B1 + S] = b1[blk] / s1
        pkbB[0, BB_B2:BB_B2 + ACTD] = bias2
        # partition-major packs: per-partition rows are contiguous in
        # DRAM so each DMA descriptor is a 4-6KB run (vs 1KB per K-row)
        a0p = A0[:XH, blk].reshape(KC0, 128, S).transpose(1, 0, 2) \
                          .reshape(128, KC0 * S)
        a1p = (a1blk / s1).reshape(KC1, 128, S).transpose(1, 0, 2) \
                          .reshape(128, KC1 * S)
        a2p = A2[blk, :].reshape(S // 128, 128, ACTD).transpose(1, 0, 2) \
                        .reshape(128, (S // 128) * ACTD)
        in_maps.append(dict(
            ctb=ctb.astype(bf), pkb=pkbk.astype(bf),
            pkbB=pkbB.astype(bf),
            a0=np.ascontiguousarray(a0p).astype(bf),
            wtail=wtail_for(blk).astype(bf),
            a1=np.ascontiguousarray(a1p).astype(f8),
            a2=np.ascontiguousarray(a2p).astype(bf),
        ))
    return in_maps


_NC_CACHE: dict = {}


def kernel(**inputs) -> np.ndarray:
    if "nc" not in _NC_CACHE:
        _NC_CACHE["nc"] = build_nc(reps=1)
    nc = _NC_CACHE["nc"]
    in_maps = make_in_maps(inputs)
    res = bass_utils.run_bass_kernel_spmd(nc, in_maps,
                                          core_ids=list(range(NCORES)))
    y = np.sum([res.results[k]["y"] for k in range(NCORES)], axis=0)
    return y.astype(np.float32)


# revision 68
# speedup vs baseline: 1.0928x; 1.0928x over previous
"""Trainium2 Bass kernel for nn_FCNetwork3D (batch-1 dense CNN+MLP).

Network: x[1,2264] -> 6x Conv3d(1,1,3,SAME)+ReLU on the 6x6x6 tail ->
concat -> normalize -> Linear(2264,4096)+tanh -> Linear(4096,4096)+tanh
-> Linear(4096,32) -> scale/shift.

Sharding (8 cores): tensor-parallel on the two wide Linears.
  L0 column-parallel: core k computes h0 block k [512] (weights pre-
    transposed + normalization folded on host), tanh locally.
  AllGather h0 (1KB/core bf16) on-device.
  L1 column-parallel: core k computes h1 block k [512], tanh locally.
  L2 row-parallel over h1 blocks: core k computes a partial [1,32]
    (out_scale folded into weights, bias/out_shift split /8 across
    cores); host unshard = sum of the 8 partials.

Precision: weights/activations bf16 (fp32 PSUM accumulate), and the
dominant a1 stream (W1.T block, 2M params/core) in fp8-e3m4 with one
bf16-rounded dequant scale per core block, folded into the h1 tanh
(bias pre-divided).  Measured end-to-end max rel err 1.22e-2 vs the
fp32 reference (gate 2e-2); hardware matches the numpy quantization
model exactly.  e4m3 fails (2.7e-2); quantizing a0 too fails (1.97e-2).

The conv stack uses the z-banded form of the 3x3x3 SAME conv on the
6x6x6 grid: 3 matrices [36,36] per layer (one per z-offset) acting on
v viewed as [36 xy-sites, 6 z-slices] — 45KB of conv weights instead
of 1.1MB of dense [216,216] matrices.

Pipelining (reps>1): on this platform each collective_compute costs
~70us of serialized wall time and utterly dominates the un-batched
per-rep cost (measured: removing the collective makes 15 extra reps
free).  So reps are processed in groups of G: phase A runs G conv+L0
blocks, ONE AllGather moves all G h0 blocks (payload is tiny either
way), then phase B runs G [L1,L2] blocks.  Groups are software-
pipelined with PER-REP interleave — emission order is
  A_0, g_0, [A_1 rep i alternating with B_0 rep i], g_1, ...
— so gather k overlaps A_{k+1}'s compute + DMA, and the a0/a1 weight
streams hit the shared DMA-engine pool in 1:1 need-order (full phase
skew let one phase's backlog delay the other's urgent tiles).

DMA plan: one light queue (sync) carries the small latency-critical
tensors (conv weights, packs, biases, gather in/out rows, y); three
bulk queues (scalar, vector, gpsimd) each carry ~2MB/rep of a0+a1 in
need-order.
"""

import numpy as np

import concourse.bass as bass
import concourse.mybir as mybir
import concourse.tile as tile
from concourse import bacc
from concourse import bass_utils

F32 = mybir.dt.float32
BF16 = mybir.dt.bfloat16
F8 = mybir.dt.float8e3      # e3m4: 4 mantissa bits; a1 weight stream
AF = mybir.ActivationFunctionType

NCORES = 8
OBS, ACTD, H, VOX = 2264, 32, 4096, 216
XH = OBS - VOX            # 2048 (x head)
S = H // NCORES           # 512 (per-core block of the hidden dim)
KC0 = XH // 128           # 16 x-head K-chunks
KC1 = H // 128            # 32 h0 K-chunks
NS = 36                   # xy-sites per z-slice
NZ = 6                    # z-slices
CTW = 6 * 3 * NS          # packed banded conv width (648)

# pkb (bf16 [128, 566] pack) column map: x chunks, voxel tail, then the
# phase-A bias row (1.0 + bias0) rides along in partition 0
PB_X = 0                  # [0:16)    x head, partition-major chunks
PB_V = 16                 # [16:22)   voxel tail as [36, 6]
BA_ONE = 22               # [22]      1.0 (partition 0)
BA_B0 = 54                # [54:566)  bias0 block (partition 0)
PB_W = 566
# ctb carries the conv bias columns (bf16) after the 18 band matrices
CF_B = CTW                # [648:654) per-layer conv bias, broadcast rows
CT_W = CTW + 6
# pkbB (bf16 [1, 576], partition 0): 1.0, fp8 dequant scale, bias1
# block (pre-divided by the scale) at 32..544, bias2 at 544
BB_ONE = 0
BB_SC = 1
BB_B1 = 32
BB_B2 = 544
BB_W = 576

# engine roles: sync + gpsimd issue only prefetch DMA (no compute, no
# dependency waits -> deep in-order lookahead); scalar runs the serial
# activation/copy chains plus the few dependency-gated DMAs (ccin,
# ccrow, y); PE computes; DVE paces the warmup/bridge chains.
A1Q = (8, 12, 12)         # a1 K-chunk split across (sync, gpsimd, gpsimd)


def build_nc(reps: int = 1, fake_gather: bool = False, G: int = 8):
    """Build the per-core Bass program (identical on all 8 cores; data
    differs via per-core inputs). reps>1 unrolls the whole body in
    gather-batched groups of up to G. fake_gather replaces the
    AllGather with a DRAM round-trip (single-core TimelineSim oracle /
    HW no-collective A/B)."""
    nc = bacc.Bacc("TRN2", target_bir_lowering=False, debug=False,
                   num_devices=1 if fake_gather else NCORES)

    ctb_d = nc.dram_tensor("ctb", [NS, CT_W], BF16, kind="ExternalInput")
    pkb_d = nc.dram_tensor("pkb", [128, PB_W], BF16, kind="ExternalInput")
    pkbB_d = nc.dram_tensor("pkbB", [1, BB_W], BF16, kind="ExternalInput")
    a0_d = nc.dram_tensor("a0", [128, KC0 * S], BF16, kind="ExternalInput")
    wtail_d = nc.dram_tensor("wtail", [NS, NZ * S], BF16, kind="ExternalInput")
    a1_d = nc.dram_tensor("a1", [128, KC1 * S], F8, kind="ExternalInput")
    a2_d = nc.dram_tensor("a2", [128, (S // 128) * ACTD], BF16, kind="ExternalInput")
    y_d = nc.dram_tensor("y", [1, ACTD], F32, kind="ExternalOutput")

    scr = nc.alloc_sbuf_tensor("warm_scr", [1, 64], BF16)

    groups = []
    r = 0
    while r < reps:
        groups.append(list(range(r, min(r + G, reps))))
        r += G

    with tile.TileContext(nc) as tc:
        with (
            tc.tile_pool(name="wa", bufs=3) as wa,
            tc.tile_pool(name="wb", bufs=2) as wb,
            tc.tile_pool(name="cp", bufs=2) as cp,
            tc.tile_pool(name="sp", bufs=2) as sp,
            tc.tile_pool(name="ps", bufs=1, space="PSUM") as ps,
            tc.tile_pool(name="dr", bufs=2, space="DRAM") as dr,
        ):
            def emit_A_rep(ccin, i, warmup: bool):
                """One phase-A rep: stage streams + conv stack + L0 +
                tanh + write h0s into the group's ccin row i."""
                if True:
                    first_group = warmup
                    # --- light HWDGE smalls, in conv/L0 need-order ---
                    # (deep-buffered: these gate in-order queues at rep
                    # boundaries if their slots recycle too slowly)
                    pkb = sp.tile([128, PB_W], BF16, name="pkb", bufs=4)
                    nc.sync.dma_start(out=pkb[:], in_=pkb_d.ap())
                    ctb = cp.tile([NS, CT_W], BF16, name="ctb", bufs=4)
                    nc.sync.dma_start(out=ctb[:], in_=ctb_d.ap())
                    wtail = cp.tile([NS, NZ * S], BF16, name="wtail", bufs=4)
                    nc.gpsimd.dma_start(out=wtail[:], in_=wtail_d.ap())

                    # --- a0 stream: one 2MB prefetch on sync ---
                    a0t = wa.tile([128, KC0 * S], BF16, name="a0t")
                    nc.sync.dma_start(out=a0t[:], in_=a0_d[:, :])

                    if first_group:
                        # tensor-clock warmup from t=0: self-paced PE<->DVE
                        # chain on an uninitialized scratch tensor.
                        pwm = ps.tile([1, 64], F32, name="ptr2", bufs=2)
                        wseed = sp.tile([1, 1], BF16, name="brb")
                        nc.vector.tensor_copy(wseed[:], scr.ap()[0:1, 0:1])
                        for w in range(6):
                            nc.tensor.matmul(pwm[:], wseed[:], scr.ap()[0:1, :],
                                             start=(w == 0), stop=(w == 5),
                                             skip_group_check=True)
                            if w < 5:
                                wseed = sp.tile([1, 1], BF16, name="brb")
                                nc.vector.tensor_copy(wseed[:], pwm[0:1, 0:1])

                    # --- conv stack + L0, interleaved ---
                    # The 6 conv relu round-trips (PE -> scalar -> PE)
                    # would stall the in-order PE ~600ns each; instead
                    # the L0 x-chunk matmuls are dealt between conv
                    # layers so PE streams weights while each relu
                    # flies.  The conv-dependent z-tail matmuls close
                    # the ph0 accumulation.
                    hp = tc.high_priority()
                    hp.__enter__()
                    one_a = pkb[0:1, BA_ONE:BA_ONE + 1]
                    ph0 = ps.tile([1, S], F32, name="ph0", bufs=2)
                    nc.tensor.matmul(ph0[:], one_a,
                                     pkb[0:1, BA_B0:BA_B0 + S],
                                     start=True, stop=False)
                    XBATCH = (3, 3, 3, 3, 2, 2)
                    v = pkb[0:NS, PB_V:PB_V + NZ]
                    c = 0
                    for li in range(6):
                        b = li * 3 * NS
                        pm = ps.tile([NS, NZ], F32, name="pm", bufs=2)
                        nc.tensor.matmul(pm[:, 0:6], ctb[:, b + 36:b + 72],
                                         v[:, 0:6], start=True, stop=False,
                                         skip_group_check=True)
                        nc.tensor.matmul(pm[:, 1:6], ctb[:, b:b + 36],
                                         v[:, 0:5], start=False, stop=False,
                                         skip_group_check=True)
                        nc.tensor.matmul(pm[:, 0:5], ctb[:, b + 72:b + 108],
                                         v[:, 1:6], start=False, stop=True,
                                         skip_group_check=True)
                        nv = sp.tile([NS, NZ], BF16, name="nv")
                        nc.scalar.activation(nv[:], pm[:], AF.Relu,
                                             bias=ctb[:, CF_B + li:CF_B + li + 1])
                        v = nv[:]
                        for _ in range(XBATCH[li]):
                            nc.tensor.matmul(ph0[:], pkb[:, c:c + 1],
                                             a0t[:, c * S:(c + 1) * S],
                                             start=False, stop=False)
                            c += 1
                    for z in range(NZ):
                        nc.tensor.matmul(ph0[:], v[:, z:z + 1],
                                         wtail[:, z * S:(z + 1) * S],
                                         start=False, stop=(z == NZ - 1))
                    h0s = sp.tile([1, S], BF16, name="h0s")
                    nc.scalar.activation(h0s[:], ph0[:], AF.Tanh)
                    nc.scalar.dma_start(out=ccin[i:i + 1, :], in_=h0s[:])
                    hp.__exit__(None, None, None)

            def emit_gather(ccin, gl):
                ccout = dr.tile([gl * H], BF16, name="ccout",
                                addr_space="Local" if fake_gather else "Shared")
                if fake_gather:
                    # stand-in: a DRAM bounce keeps the dependency chain
                    # identical to the real AllGather
                    nc.gpsimd.dma_start(
                        out=ccout[0:gl * S],
                        in_=ccin[:].rearrange("o e -> (o e)"))
                else:
                    nc.gpsimd.collective_compute(
                        "AllGather", mybir.AluOpType.bypass,
                        replica_groups=[list(range(NCORES))],
                        ins=[ccin[:].rearrange("o e -> (o e)").opt()],
                        outs=[ccout[:].opt()])
                return ccout

            def emit_B_rep(ccout, gl, ii):
                """One phase-B rep: fetch+transpose h0, L1, L2 partial,
                y.  (No p-state bridge: with batched gathers the PE
                never idles long enough to down-clock, and the chain
                serialized the in-order engines against real work.)"""
                if True:
                    pkbB = sp.tile([1, BB_W], BF16, name="pkbB")
                    nc.sync.dma_start(out=pkbB[:], in_=pkbB_d.ap())
                    a2t = sp.tile([128, (S // 128) * ACTD], BF16, name="a2t")
                    nc.sync.dma_start(out=a2t[:], in_=a2_d[:, :])
                    one_b = pkbB[0:1, BB_ONE:BB_ONE + 1]
                    # fp8 dequant scale, widened to f32 for the Act unit
                    sc32 = sp.tile([1, 1], F32, name="sc32")
                    nc.vector.tensor_copy(sc32[:], pkbB[0:1, BB_SC:BB_SC + 1])

                    # this rep's gathered h0 row: one strided DMA pulling
                    # the 8 [1,512] core-blocks out of the group buffer
                    ccrow = sp.tile([1, H], BF16, name="ccrow")
                    nc.scalar.dma_start(
                        out=ccrow[:],
                        in_=ccout.rearrange("(c g) -> c g", c=NCORES)
                                 [:, ii * S:(ii + 1) * S])

                    # --- a1 stream: sync + 2x gpsimd prefetches ---
                    a1eng = [nc.sync, nc.gpsimd, nc.gpsimd]
                    a1t = []
                    c1 = 0
                    for q, gch in enumerate(A1Q):
                        wt = wb.tile([128, gch * S], F8, name=f"a1q{q}")
                        a1eng[q].dma_start(
                            out=wt[:], in_=a1_d[:, c1 * S:(c1 + gch) * S])
                        a1t.append(wt)
                        c1 += gch

                    # --- h0 row -> K-chunk columns (PE transposes) ---
                    h0g = sp.tile([128, KC1], BF16, name="h0g")
                    for t in range(4):
                        ptr2 = ps.tile([128, 8], F32, name="ptr2", bufs=2)
                        for j in range(8):
                            cc = t * 8 + j
                            nc.tensor.matmul(
                                ptr2[:, j:j + 1],
                                ccrow[:, cc * 128:(cc + 1) * 128], one_b,
                                start=True, stop=True)
                        nc.scalar.copy(h0g[:, t * 8:(t + 1) * 8], ptr2[:])

                    # --- L1: h1_blk = tanh(h0 @ A1 + b1_blk) [1,512] ---
                    ph1 = ps.tile([1, S], F32, name="ph1")
                    nc.tensor.matmul(ph1[:], one_b,
                                     pkbB[0:1, BB_B1:BB_B1 + S],
                                     start=True, stop=False)
                    # consume the gpsimd tiles (q1, q2) first; sync's q0
                    # trails the next group's a0 stream on its queue, so
                    # its chunks go last in the accumulation
                    qbase = [0, A1Q[0], A1Q[0] + A1Q[1]]
                    qorder = [0, 1, 2]
                    n = 0
                    for q in qorder:
                        for j in range(A1Q[q]):
                            cc = qbase[q] + j
                            n += 1
                            nc.tensor.matmul(ph1[:], h0g[:, cc:cc + 1],
                                             a1t[q][:, j * S:(j + 1) * S],
                                             start=False, stop=(n == KC1))
                    h1s = sp.tile([1, S], BF16, name="h1s")
                    nc.scalar.activation(h1s[:], ph1[:], AF.Tanh,
                                         scale=sc32[:])

                    # --- L2 partial: y_k = h1_blk @ A2_blk + b'/8 ---
                    pth = ps.tile([128, S // 128], F32, name="ptr2", bufs=2)
                    for ccq in range(S // 128):
                        nc.tensor.matmul(pth[:, ccq:ccq + 1],
                                         h1s[:, ccq * 128:(ccq + 1) * 128],
                                         one_b, start=True, stop=True)
                    h1g = sp.tile([128, S // 128], BF16, name="h1g")
                    nc.scalar.copy(h1g[:], pth[:])
                    py = ps.tile([1, ACTD], F32, name="py")
                    nc.tensor.matmul(py[:], one_b,
                                     pkbB[0:1, BB_B2:BB_B2 + ACTD],
                                     start=True, stop=False)
                    for ccq in range(S // 128):
                        nc.tensor.matmul(py[:], h1g[:, ccq:ccq + 1],
                                         a2t[:, ccq * ACTD:(ccq + 1) * ACTD],
                                         start=False,
                                         stop=(ccq == S // 128 - 1))
                    ys = sp.tile([1, ACTD], F32, name="ys")
                    nc.scalar.copy(ys[:], py[:])
                    nc.sync.dma_start(out=y_d[:, :], in_=ys[:])

            # ---- software pipeline over groups, rep-interleaved:
            # A_{k+1} rep i and B_k rep i alternate, so the a0 and a1
            # streams hit the shared DMA pool in 1:1 need-order ----
            gl0 = len(groups[0])
            ccin = dr.tile([gl0, S], BF16, name="ccin")
            for i in range(gl0):
                emit_A_rep(ccin, i, warmup=(i == 0))
            pend = (emit_gather(ccin, gl0), gl0)
            for k in range(1, len(groups)):
                glk = len(groups[k])
                ccin = dr.tile([glk, S], BF16, name="ccin")
                pout, pgl = pend
                for i in range(max(glk, pgl)):
                    if i < glk:
                        emit_A_rep(ccin, i, warmup=False)
                    if i < pgl:
                        emit_B_rep(pout, pgl, i)
                pend = (emit_gather(ccin, glk), glk)
            pout, pgl = pend
            for ii in range(pgl):
                emit_B_rep(pout, pgl, ii)

    nc.compile()
    return nc


def _conv_matrix(w: np.ndarray) -> np.ndarray:
    """[216,216] dense matrix of a 3x3x3 SAME cross-correlation on a
    6x6x6 grid: C[o, i] such that y.flat = C @ v.flat."""
    w = np.asarray(w, dtype=np.float32).reshape(3, 3, 3)
    C = np.zeros((VOX, VOX), dtype=np.float32)
    idx = np.arange(6)
    for dz in (-1, 0, 1):
        for dy in (-1, 0, 1):
            for dx in (-1, 0, 1):
                zo, zi = idx[max(0, -dz):6 - max(0, dz)], idx[max(0, dz):6 - max(0, -dz)]
                yo, yi = idx[max(0, -dy):6 - max(0, dy)], idx[max(0, dy):6 - max(0, -dy)]
                xo, xi = idx[max(0, -dx):6 - max(0, dx)], idx[max(0, dx):6 - max(0, -dx)]
                o = (zo[:, None, None] * 36 + yo[None, :, None] * 6 + xo[None, None, :]).ravel()
                i = (zi[:, None, None] * 36 + yi[None, :, None] * 6 + xi[None, None, :]).ravel()
                C[o, i] = w[dz + 1, dy + 1, dx + 1]
    return C


def make_in_maps(inputs: dict) -> list[dict]:
    """Host-side layout prep + sharding: fold normalization into A0,
    out_scale/shift into A2, pre-transpose weights, build banded conv
    matrices, quantize everything to bf16."""
    import ml_dtypes
    f = np.float32
    bf = ml_dtypes.bfloat16
    f8 = mybir.dt.np(F8)
    F8MAX = float(ml_dtypes.finfo(f8).max)
    x = np.asarray(inputs["x"], f)
    W0, b0 = np.asarray(inputs["W0"], f), np.asarray(inputs["b0"], f)
    W1, b1 = np.asarray(inputs["W1"], f), np.asarray(inputs["b1"], f)
    W2, b2 = np.asarray(inputs["W2"], f), np.asarray(inputs["b2"], f)
    in_shift = np.asarray(inputs["in_shift"], f)
    in_scale = np.asarray(inputs["in_scale"], f)
    out_shift = np.asarray(inputs["out_shift"], f)
    out_scale = np.asarray(inputs["out_scale"], f)

    sc = (1.0 / (in_scale.astype(np.float64) + 1e-8)).astype(f)       # [2264]
    A0 = (W0 * sc[None, :]).T.astype(f)                               # [2264, 4096]
    bias0 = (b0 - (in_shift * sc) @ W0.T).astype(f)                   # [4096]
    A1 = W1.T.astype(f)                                               # [4096, 4096]
    A2 = (W2.T * out_scale[None, :]).astype(f)                        # [4096, 32]
    bias2 = ((b2 * out_scale + out_shift) / NCORES).astype(f)         # [32]

    # banded conv: per layer i and z-offset dz, M_dz [36,36] stored
    # transposed (lhsT layout): ctb[s_in, (i*3 + dz+1)*36 + s_out];
    # per-layer conv biases ride along as broadcast columns at CF_B
    ctb = np.zeros((NS, CT_W), f)
    for i in range(6):
        C = _conv_matrix(inputs[f"cw{i}"])
        for dzi, dz in enumerate((-1, 0, 1)):
            M = C[1 * NS:2 * NS, (1 + dz) * NS:(2 + dz) * NS]         # [out, in]
            ctb[:, (i * 3 + dzi) * NS:(i * 3 + dzi + 1) * NS] = M.T
    cb = np.array([np.asarray(inputs[f"cb{i}"], f).ravel()[0]
                   for i in range(6)], f)
    ctb[:, CF_B:CF_B + 6] = np.broadcast_to(cb[None, :], (NS, 6))

    xh = x.ravel()[:XH]
    v0 = x.ravel()[XH:]
    pkb = np.zeros((128, PB_W), f)
    pkb[:, PB_X:PB_X + KC0] = xh.reshape(KC0, 128).T
    pkb[0:NS, PB_V:PB_V + NZ] = v0.reshape(NZ, NS).T
    pkb[0, BA_ONE] = 1.0  # bias0 row is filled per-core below

    # A0 tail columns regrouped by z-slice: wtail[s, z*S+n] = A0[2048+z*36+s, n]
    def wtail_for(blk):
        t = A0[XH:OBS, blk].reshape(NZ, NS, S)
        return np.ascontiguousarray(t.transpose(1, 0, 2).reshape(NS, NZ * S))

    in_maps = []
    for k in range(NCORES):
        blk = slice(k * S, (k + 1) * S)
        pkbk = pkb.copy()
        pkbk[0, BA_B0:BA_B0 + S] = bias0[blk]
        a1blk = A1[:, blk]
        # fp8-e3m4 a1: one bf16-rounded dequant scale per core block,
        # folded into the h1 tanh (bias pre-divided so it scales back)
        s1 = np.float32(np.asarray(np.abs(a1blk).max() / F8MAX, bf))
        pkbB = np.zeros((1, BB_W), f)
        pkbB[0, BB_ONE] = 1.0
        pkbB[0, BB_SC] = s1
        pkbB[0, BB_B1:BB_B1 + S] = b1[blk] / s1
        pkbB[0, BB_B2:BB_B2 + ACTD] = bias2
        # partition-major packs: per-partition rows are contiguous in
        # DRAM so each DMA descriptor is a 4-6KB run (vs 1KB per K-row)
        a0p = A0[:XH, blk].reshape(KC0, 128, S).transpose(1, 0, 2) \
                          .reshape(128, KC0 * S)
        a1p = (a1blk / s1).reshape(KC1, 128, S).transpose(1, 0, 2) \
                          .reshape(128, KC1 * S)
        a2p = A2[blk, :].reshape(S // 128, 128, ACTD).transpose(1, 0, 2) \
                        .reshape(128, (S // 128) * ACTD)
        in_maps.append(dict(
            ctb=ctb.astype(bf), pkb=pkbk.astype(bf),
            pkbB=pkbB.astype(bf),
            a0=np.ascontiguousarray(a0p).astype(bf),
            wtail=wtail_for(blk).astype(bf),
            a1=np.ascontiguousarray(a1p).astype(f8),
            a2=np.ascontiguousarray(a2p).astype(bf),
        ))
    return in_maps


_NC_CACHE: dict = {}


def kernel(**inputs) -> np.ndarray:
    if "nc" not in _NC_CACHE:
        _NC_CACHE["nc"] = build_nc(reps=1)
    nc = _NC_CACHE["nc"]
    in_maps = make_in_maps(inputs)
    res = bass_utils.run_bass_kernel_spmd(nc, in_maps,
                                          core_ids=list(range(NCORES)))
    y = np.sum([res.results[k]["y"] for k in range(NCORES)], axis=0)
    return y.astype(np.float32)
